# revision 18
# baseline (speedup 1.0000x reference)
"""CQAttention (QANet context-query attention) Trainium2 Bass kernel.

Full-input contract: kernel(C, Q, cmask, qmask, w) -> (B, 4D, LC) f32.
Shards batch B=16 across 8 NeuronCores (2 examples/core), runs one SPMD
Bass/Tile program, gathers results.

Math (per example, d=512, Lc=2048, Lq=512):
  S = Cb@w1 [i] + Qb@w2 [j] + (Cb*w3)@Qb^T          (Lc, Lq)
  S1 = softmax_j(S), S2 = softmax_i(S)
  A = S1@Qb ; Bt = S1@S2^T@Cb
  out = concat([Cb, A, Cb*A, Cb*Bt], feat).T        (4d, Lc)

Kernel structure (all layouts "feature-on-partitions" = input layout of
C/Q = required output layout):
  - ONE exp array serves both softmaxes: E = exp(S) with the full
    trilinear logits (w1 folded into Qmod per-partition, the rank-1
    w2-term added via a K=1 ones x c2row matmul into the same PSUM
    group). Each softmax's invariant shift cancels in its own
    normalization, so no max-subtraction is needed (|S| <= ~8 for
    N(0,1)-scale inputs).
  - matmul operands are bf16 (PSUM accumulates fp32): full 1 col/cycle
    PE rate, fast-weight-load on LDWEIGHTS, half the SBUF traffic.
  - rowsum[i] (S1 normalizer) comes FREE from the exp activation's
    accum_out. The S1 normalization is folded into the E->E1T
    transposes by using diag(1/rowsum-chunk) as the transpose's moving
    operand, so E1T comes out of PSUM fully normalized (no [128,512]
    reciprocals, no repmat, no in-place normalize pass). The
    transposes live INSIDE the stream loop, trailing their exp by one
    chunk, so there is no serial transpose phase at the end.
  - ssum[j] (S2 normalizer) via ones-vector matmuls; its reciprocal is
    transposed to per-partition columns first so the DVE reciprocal is
    [128,4] instead of [1,512].
  - A^T = Qbt @ E1T, Bt^T = T2s @ E1T; o3/o4 output rows multiply the
    PSUM results directly against the resident bf16 C tile (no fp32 C
    reload in phase E).
  - C chunk loads + fp32->bf16 casts for example n+1 are deferred into
    example n's output phase, interleaved one per output block and
    split between ACT and DVE (never GpSimd: its SBUF traffic
    collides with DVE tensor_scalar and serializes both).
"""

import numpy as np

import concourse.bass as bass
import concourse.tile as tile
from concourse import bacc, mybir
from concourse.bass_utils import run_bass_kernel_spmd
from concourse.masks import make_identity

B, D, LC, LQ = 16, 512, 2048, 512
NCORES = 8
BL = B // NCORES  # examples per core
KD = D // 128  # 4 d-chunks
KJ = LQ // 128  # 4 j-chunks
NI = LC // 512  # 4 i column-chunks
MI = LC // 128  # 16 i partition-chunks

F32 = mybir.dt.float32
BF16 = mybir.dt.bfloat16
FP8 = mybir.dt.float8e4
DR = mybir.MatmulPerfMode.DoubleRow
EXP = mybir.ActivationFunctionType.Exp
COPY = mybir.ActivationFunctionType.Copy
MUL = mybir.AluOpType.mult
ADD = mybir.AluOpType.add


class Ctx:
    pass


def _pools(tc, ctx):
    P = Ctx()
    P.const = ctx.enter_context(tc.tile_pool(name="const", bufs=1))
    P.cstage = ctx.enter_context(tc.tile_pool(name="cstage", bufs=4))
    P.qt = ctx.enter_context(tc.tile_pool(name="qt", bufs=1))
    P.big = ctx.enter_context(tc.tile_pool(name="big", bufs=1))
    P.mid = ctx.enter_context(tc.tile_pool(name="mid", bufs=2))
    P.stream = ctx.enter_context(tc.tile_pool(name="stream", bufs=1))
    P.ost = ctx.enter_context(tc.tile_pool(name="ost", bufs=2))
    P.psum = ctx.enter_context(tc.tile_pool(name="psum", space="PSUM", bufs=8))
    return P


def _c_dma(nc, P, Cd, Od, b, k):
    """DMA one C d-chunk in and copy it to the output C rows."""
    cst = P.cstage.tile([128, LC], F32, tag="cstage", name=f"cst{b}_{k}")
    nc.sync.dma_start(out=cst, in_=Cd[b, k * 128 : (k + 1) * 128, :])
    nc.sync.dma_start(out=Od[b, k * 128 : (k + 1) * 128, :], in_=cst)
    return cst


def _c_cast(nc, T, cst, k):
    """Cast one C d-chunk to bf16. Chunks 0,2 on ACT; 1,3 on DVE."""
    if k % 2 == 0:
        nc.scalar.activation(T.CtR[:, k, :], cst, COPY)
    else:
        nc.vector.tensor_copy(T.CtR[:, k, :], cst)


def _c_chunk_load(nc, P, T, Cd, Od, b, k):
    _c_cast(nc, T, _c_dma(nc, P, Cd, Od, b, k), k)


def _phase_A(nc, P, K, T, Cd, Qd, Od, b, load_c):
    """Q loads, bf16 casts, Q transpose, c2 row (the w2 rank-1 term).
    For the first example (load_c) the C DMAs are issued up front and
    the casts follow the Q casts, so the CDT stream's first matmul
    group has all four CtR chunks as early as possible."""
    psum = P.psum
    Qt = P.qt.tile([128, KD, LQ], F32, tag="qt", name=f"qt{b}")
    nc.sync.dma_start(out=Qt, in_=Qd[b].rearrange("(a p) j -> p a j", p=128))
    csts = [_c_dma(nc, P, Cd, Od, b, k) for k in range(KD)] if load_c else None
    # wsb cols: 0-3 w1, 4-7 w2, 8-11 w3
    for k in range(KD):
        nc.scalar.activation(T.QtB[:, k, :], Qt[:, k, :], COPY)
        nc.vector.tensor_scalar(
            out=T.Qmod[:, k, :], in0=Qt[:, k, :],
            scalar1=K.wsb[:, 8 + k : 9 + k], scalar2=K.wsb[:, k : k + 1],
            op0=MUL, op1=ADD,
        )
    if csts is not None:
        for k in range(KD):
            _c_cast(nc, T, csts[k], k)
    qps = [
        psum.tile([128, D], BF16, tag="ps", name=f"qps{b}_{c}") for c in range(KJ)
    ]
    for a in range(KD):
        for c in range(KJ):
            nc.tensor.transpose(
                qps[c][:, a * 128 : (a + 1) * 128],
                T.QtB[:, a, c * 128 : (c + 1) * 128],
                K.identB,
            )
    for c in range(KJ):
        nc.vector.tensor_copy(T.Qbt[:, c, :], qps[c])
    # c2[j] = Q^T w2 as a bf16 row; enters the e2 PSUM group as ones x c2row
    c2ps = psum.tile([1, LQ], F32, tag="ps", name=f"c2ps{b}")
    for kd in range(KD):
        nc.tensor.matmul(
            c2ps, K.wsbB[:, 4 + kd : 5 + kd], T.QtB[:, kd, :],
            start=(kd == 0), stop=(kd == KD - 1),
        )
    nc.scalar.activation(T.c2rowB, c2ps, COPY)


def _phase_CDT(nc, P, K, T, b):
    """Stream i-chunks: E=exp(logits) (+rowsum accum) -> trailing
    normalized transposes into E1T + T2/ssum accumulation. Consumers
    trail the exp by one chunk so PE never waits on ACT."""
    psum = P.psum
    t2ps = [
        psum.tile([128, D], F32, tag="ps", name=f"t2ps{b}_{m}") for m in range(KJ)
    ]
    ssps = psum.tile([1, LQ], F32, tag="ps", name=f"ssps{b}")
    T.t2ps, T.ssps = t2ps, ssps
    cbt_sbs = {}

    def trailing(ki):
        isl = slice(ki * 128, (ki + 1) * 128)
        nc.vector.reciprocal(T.rec1c[:, ki : ki + 1], T.rsraw[:, ki : ki + 1])
        diag = P.stream.tile(
            [128, 128], BF16, tag="diag", bufs=3, name=f"dg{b}_{ki}"
        )
        nc.vector.tensor_scalar(
            out=diag, in0=K.identB, scalar1=T.rec1c[:, ki : ki + 1],
            scalar2=None, op0=MUL,
        )
        tr_ps = psum.tile([128, KJ, 128], F32, tag="ps", name=f"trps{b}_{ki}")
        for kj in range(KJ):
            nc.tensor.matmul(
                tr_ps[:, kj, :],
                T.E2[:, ki, kj * 128 : (kj + 1) * 128],
                diag,
                start=True, stop=True,
            )
        nc.scalar.activation(T.E1T[:, :, isl], tr_ps, COPY)
        nc.tensor.matmul(
            ssps, K.ones_colB, T.E2[:, ki, :], start=(ki == 0), stop=(ki == MI - 1)
        )
        for mj in range(KJ):
            nc.tensor.matmul(
                t2ps[mj], T.E2[:, ki, mj * 128 : (mj + 1) * 128],
                cbt_sbs.pop(ki) if mj == KJ - 1 else cbt_sbs[ki],
                start=(ki == 0), stop=(ki == MI - 1),
            )

    for ki in range(MI):
        isl = slice(ki * 128, (ki + 1) * 128)
        cbt_ps = psum.tile([128, D], BF16, tag="ps", name=f"cps{b}_{ki}")
        for kd in range(KD):
            nc.tensor.transpose(
                cbt_ps[:, kd * 128 : (kd + 1) * 128], T.CtR[:, kd, isl], K.identB
            )
        cbt_sb = P.stream.tile(
            [128, D], BF16, tag="cbt", bufs=3, name=f"cbt{b}_{ki}"
        )
        nc.vector.tensor_copy(cbt_sb, cbt_ps)
        cbt_sbs[ki] = cbt_sb

        e2ps = psum.tile([128, LQ], F32, tag="ps", name=f"e2ps{b}_{ki}")
        for kd in range(KD):
            nc.tensor.matmul(
                e2ps, T.CtR[:, kd, isl], T.Qmod[:, kd, :],
                start=(kd == 0), stop=False,
            )
        nc.tensor.matmul(e2ps, K.ones_rowB, T.c2rowB, start=False, stop=True)
        nc.scalar.activation(
            T.E2[:, ki, :], e2ps, EXP, accum_out=T.rsraw[:, ki : ki + 1]
        )
        if ki > 0:
            trailing(ki - 1)
    trailing(MI - 1)


def _phase_R(nc, P, K, T, b):
    """S2 normalizer: transpose ssum to columns, tiny reciprocal, scale
    T2 into bf16."""
    ssrow = P.mid.tile([1, LQ], F32, tag="ssrow", name=f"ssr{b}")
    nc.scalar.activation(ssrow, T.ssps, COPY)
    sscol_ps = P.psum.tile([128, KJ], F32, tag="ps", name=f"sscps{b}")
    for jm in range(KJ):
        nc.tensor.transpose(
            sscol_ps[:, jm : jm + 1],
            ssrow[:, jm * 128 : (jm + 1) * 128],
            K.ident[:1, :1],
        )
    rec2col = P.mid.tile([128, KJ], F32, tag="rec2col", name=f"r2c{b}")
    nc.vector.reciprocal(rec2col, sscol_ps)
    for mj in range(KJ):
        nc.vector.tensor_scalar(
            out=T.T2s[:, mj, :], in0=T.t2ps[mj],
            scalar1=rec2col[:, mj : mj + 1], scalar2=None, op0=MUL,
        )


def _phase_E(nc, P, K, T, Od, b, deferred=()):
    """A^T, C*A^T, C*Bt^T (rows d, cols i). E1T is pre-normalized, so
    A/Bt matmuls need no further scaling: o2 is an ACT copy, o3/o4 are
    single DVE multiplies of the PSUM tiles with bf16 C rows. One
    deferred closure (next example's C-chunk load) runs per block."""
    psum = P.psum
    deferred = list(deferred)
    for md in range(4):
        msl = slice(md * 128, (md + 1) * 128)
        for h in range(2):
            hsl = slice(h * 1024, (h + 1) * 1024)
            o2 = P.ost.tile([128, 1024], F32, tag="o2", name=f"o2_{b}_{md}_{h}")
            o3 = P.ost.tile([128, 1024], F32, tag="o3", name=f"o3_{b}_{md}_{h}")
            for ni in (2 * h, 2 * h + 1):
                nsl = slice(ni * 512, (ni + 1) * 512)
                osl = slice((ni - 2 * h) * 512, (ni - 2 * h + 1) * 512)
                aps = psum.tile([128, 512], F32, tag="ps", name=f"aps{b}_{md}_{ni}")
                for kj in range(KJ):
                    nc.tensor.matmul(
                        aps, T.Qbt[:, kj, msl], T.E1T[:, kj, nsl],
                        start=(kj == 0), stop=(kj == KJ - 1),
                    )
                nc.scalar.activation(o2[:, osl], aps, COPY)
                nc.vector.tensor_mul(o3[:, osl], aps, T.CtR[:, md, nsl])
            if deferred:
                deferred.pop(0)()
            nc.sync.dma_start(
                out=Od[b, D + md * 128 : D + (md + 1) * 128, hsl], in_=o2
            )
            nc.sync.dma_start(
                out=Od[b, 2 * D + md * 128 : 2 * D + (md + 1) * 128, hsl], in_=o3
            )
        for h in range(2):
            hsl = slice(h * 1024, (h + 1) * 1024)
            o4 = P.ost.tile([128, 1024], F32, tag="o4", name=f"o4_{b}_{md}_{h}")
            for ni in (2 * h, 2 * h + 1):
                nsl = slice(ni * 512, (ni + 1) * 512)
                osl = slice((ni - 2 * h) * 512, (ni - 2 * h + 1) * 512)
                bps = psum.tile([128, 512], F32, tag="ps", name=f"bps{b}_{md}_{ni}")
                for kj in range(KJ):
                    nc.tensor.matmul(
                        bps, T.T2s[:, kj, msl], T.E1T[:, kj, nsl],
                        start=(kj == 0), stop=(kj == KJ - 1),
                    )
                nc.vector.tensor_mul(o4[:, osl], bps, T.CtR[:, md, nsl])
            if deferred:
                deferred.pop(0)()
            nc.sync.dma_start(
                out=Od[b, 3 * D + md * 128 : 3 * D + (md + 1) * 128, hsl], in_=o4
            )
    for fn in deferred:
        fn()


def build(bl=BL, num_devices=NCORES, enable_asserts=False):
    from contextlib import ExitStack
    from functools import partial

    nc = bacc.Bacc(
        "TRN2",
        target_bir_lowering=False,
        debug=False,
        enable_asserts=enable_asserts,
        num_devices=num_devices,
    )
    Cd = nc.dram_tensor("C", (bl, D, LC), F32, kind="ExternalInput").ap()
    Qd = nc.dram_tensor("Q", (bl, D, LQ), F32, kind="ExternalInput").ap()
    wd = nc.dram_tensor("w", (3 * D,), F32, kind="ExternalInput").ap()
    Od = nc.dram_tensor("out", (bl, 4 * D, LC), F32, kind="ExternalOutput").ap()

    with tile.TileContext(nc) as tc, ExitStack() as ctx:
        P = _pools(tc, ctx)
        K = Ctx()
        K.ident = P.const.tile([128, 128], F32, name="ident")
        make_identity(nc, K.ident)
        K.identB = P.const.tile([128, 128], BF16, name="identB")
        nc.vector.tensor_copy(K.identB, K.ident)
        ones_col_f = P.const.tile([128, 1], F32, name="ocf")
        nc.vector.memset(ones_col_f, 1.0)
        K.ones_colB = P.const.tile([128, 1], BF16, name="oc")
        nc.vector.tensor_copy(K.ones_colB, ones_col_f)
        ones_row_f = P.const.tile([1, 128], F32, name="orf")
        nc.vector.memset(ones_row_f, 1.0)
        K.ones_rowB = P.const.tile([1, 128], BF16, name="orr")
        nc.vector.tensor_copy(K.ones_rowB, ones_row_f)
        K.wsb = P.const.tile([128, 12], F32, name="wsb")
        nc.sync.dma_start(out=K.wsb, in_=wd.rearrange("(c p) -> p c", p=128))
        K.wsbB = P.const.tile([128, 12], BF16, name="wsbB")
        nc.vector.tensor_copy(K.wsbB, K.wsb)

        tiles = {}
        for b in range(bl):
            T = tiles[b] = Ctx()
            T.E2 = P.big.tile([128, MI, LQ], BF16, tag="e2", bufs=2, name=f"e2_{b}")
            T.E1T = P.big.tile([128, KJ, LC], BF16, tag="e1t", bufs=2, name=f"e1t{b}")
            T.CtR = P.big.tile([128, KD, LC], BF16, tag="ctr", bufs=2, name=f"ctr{b}")
            T.QtB = P.big.tile([128, KD, LQ], BF16, tag="qtb", bufs=2, name=f"qtb{b}")
            T.Qmod = P.big.tile([128, KD, LQ], BF16, tag="qmod", bufs=2, name=f"qm{b}")
            T.Qbt = P.big.tile([128, KJ, D], BF16, tag="qbt", bufs=2, name=f"qbt{b}")
            T.T2s = P.big.tile([128, KJ, D], BF16, tag="t2s", bufs=2, name=f"t2s{b}")
            T.c2rowB = P.big.tile([1, LQ], BF16, tag="c2row", bufs=2, name=f"c2r{b}")
            T.rsraw = P.big.tile([128, MI], F32, tag="rsraw", bufs=2, name=f"rs{b}")
            T.rec1c = P.big.tile([128, MI], F32, tag="rec1c", bufs=2, name=f"rc{b}")
            _phase_A(nc, P, K, T, Cd, Qd, Od, b, load_c=(b == 0))
            if b > 0:
                deferred = [
                    partial(_c_chunk_load, nc, P, T, Cd, Od, b, k)
                    for k in range(KD)
                ]
                _phase_E(nc, P, K, tiles[b - 1], Od, b - 1, deferred)
            _phase_CDT(nc, P, K, T, b)
            _phase_R(nc, P, K, T, b)
        _phase_E(nc, P, K, tiles[bl - 1], Od, bl - 1)
    nc.compile()
    return nc


_NC = None


def kernel(C, Q, cmask, qmask, w):
    global _NC
    C = np.ascontiguousarray(np.asarray(C, dtype=np.float32))
    Q = np.ascontiguousarray(np.asarray(Q, dtype=np.float32))
    w = np.ascontiguousarray(np.asarray(w, dtype=np.float32))
    # masks are all-ones per the problem spec; softmax masking is a no-op
    if _NC is None:
        _NC = build()
    in_maps = [
        {
            "C": np.ascontiguousarray(C[i * BL : (i + 1) * BL]),
            "Q": np.ascontiguousarray(Q[i * BL : (i + 1) * BL]),
            "w": w,
        }
        for i in range(NCORES)
    ]
    res = run_bass_kernel_spmd(_NC, in_maps, core_ids=list(range(NCORES)))
    return np.concatenate([res.results[i]["out"] for i in range(NCORES)], axis=0)
